# revision 1
# baseline (speedup 1.0000x reference)
"""Trainium2 Bass kernel for BipartiteHeteroGNN (gnn_message_passing).

Strategy (8 NeuronCores, SPMD):
- Nodes (vals/cons) sharded by id: core c owns ids [c*2500, (c+1)*2500).
- Edges assigned by destination core, sorted by dst, grouped into 128-dst
  "segment tiles"; per-edge src features fetched with dma_gather (256B rows,
  64 fp16 features + 64 pad) from a replicated node table in HBM, using
  4 SWDGE queues and small (TPB=8) groups for deep pipelining.
- fp16 end to end: node features, edge-message pipeline, MLP/encoder/pred
  weights, gather tables and AllGather payloads (fp32 only in PSUM and
  biases); rel-err stays ~5e-4, well under the 2e-2 gate.
- Segment softmax without segment-max (messages are relu(..)+eps >= 0 and
  bounded, so exp() never overflows; guard 1e-16 keeps empty segments at 0).
- Scatter-add per segment tile via one-hot matmul on the tensor engine
  (fp16 one-hots precomputed on host; fp16 ex/p values; fp32 PSUM accum).
- Node MLPs in feature-major ("T") layout so biases are per-partition;
  MLP relu / bias adds / transpose copies run on the Activation engine to
  unload the Vector engine (the busiest compute engine in this kernel).
- Cross-core exchange of updated fp16 node features via AllGather collective
  into a compact [TBL, 64] staging table, then one strided DMA re-pads rows
  to 256B for the gather (+edge-bias of the consumer layer pre-folded).

Bench notes: the harness's timed loop carries a ~2.5-3.5 ms/iter axon
dispatch floor that drifts with machine load; _run_benched also times a
trivial kernel to report a floor-calibrated kernel-only number.
"""
import numpy as np

P = 128
NCORES = 8
NV = NC = 20000
E = 500000
HID = 64
NL = 3
EPS = 1e-7
ND = 2500          # dst nodes per core (per node type)
NSEG = 20          # segment tiles per core (ceil(2500/128))
NDP = NSEG * P     # padded dst nodes per core = 2560
TBL = NCORES * NDP # gather table rows = 20480
import os as _os0
TPB = int(_os0.environ.get("GNN_TPB", "8"))   # edge tiles per gather group
GROUP = TPB * P    # edges per gather group
NQ = int(_os0.environ.get("GNN_QUEUES", "4"))  # SWDGE queues (max 4)
SBUFS = int(_os0.environ.get("GNN_SBUFS", "6"))  # stream pool bufs
WBUFS = int(_os0.environ.get("GNN_WBUFS", "4"))  # work pool bufs
OHDT = _os0.environ.get("GNN_OHDT", "f16")        # one-hot dtype: f8 | f16
TBLDT = _os0.environ.get("GNN_TBLDT", "f16")      # gather table dtype: f16 | f32
EDGE16 = _os0.environ.get("GNN_EDGE16", "1") != "0"  # f16 edge msg pipeline
NODE16 = _os0.environ.get("GNN_NODE16", "1") != "0"  # f16 node feats + MLPs
F32 = None         # set lazily (mybir)
F16 = None
I16 = None

_PROG_CACHE = {}


# ---------------------------------------------------------------- host prep

def _prep_direction(src, dst, ewt):
    """Edge preprocessing for one direction.

    Returns (per_core, schedule, ntiles):
      per_core[c] = dict(gidx [128, EC//16] i16, ewt [128, EC//128] f32,
                         oh [128, EC//128, 128] f16)
      schedule[t] = (segtile, is_start, is_stop) for each edge tile t.
    """
    src = np.asarray(src)
    dst = np.asarray(dst)
    ewt = np.asarray(ewt).reshape(-1)
    cores = []
    counts = np.zeros((NCORES, NSEG), np.int64)
    for c in range(NCORES):
        lo = c * ND
        m = (dst >= lo) & (dst < lo + ND)
        s_c = src[m]
        d_c = (dst[m] - lo).astype(np.int64)
        w_c = ewt[m]
        order = np.argsort(d_c, kind="stable")
        s_c, d_c, w_c = s_c[order], d_c[order], w_c[order]
        st_of = d_c // P
        bounds = np.searchsorted(st_of, np.arange(NSEG + 1))
        cores.append((s_c, d_c, w_c, bounds))
        counts[c] = bounds[1:] - bounds[:-1]
    st_tiles = np.maximum(1, np.ceil(counts.max(axis=0) / P).astype(np.int64))
    ntiles = int(st_tiles.sum())
    pad_tiles = (-ntiles) % TPB
    st_tiles[NSEG - 1] += pad_tiles       # merge trailing pads into last segtile
    ntiles += pad_tiles
    EC = ntiles * P

    schedule = []
    for st in range(NSEG):
        for k in range(st_tiles[st]):
            schedule.append((st, k == 0, k == st_tiles[st] - 1))

    per_core = []
    for c in range(NCORES):
        s_c, d_c, w_c, bounds = cores[c]
        src_pad = np.zeros(EC, np.int64)
        dstl_pad = np.full(EC, -1, np.int64)
        ewt_pad = np.zeros(EC, np.float32)
        pos = 0
        for st in range(NSEG):
            sl = slice(bounds[st], bounds[st + 1])
            n = bounds[st + 1] - bounds[st]
            src_pad[pos:pos + n] = s_c[sl]
            dstl_pad[pos:pos + n] = d_c[sl] - st * P
            ewt_pad[pos:pos + n] = w_c[sl]
            pos += int(st_tiles[st]) * P
        # remap src node id -> padded table row
        tbl_row = (src_pad // ND) * NDP + (src_pad % ND)
        gidx = np.tile(tbl_row.reshape(EC // 16, 16).T.astype(np.int16), (8, 1))
        ewt_t = ewt_pad.reshape(ntiles, P).T.astype(
            np.float16 if EDGE16 else np.float32)  # [128, ntiles]
        dstl2 = dstl_pad.reshape(ntiles, P).T                  # [128, ntiles]
        import ml_dtypes
        oh_dt = ml_dtypes.float8_e4m3 if OHDT == "f8" else np.float16
        oh = (dstl2[:, :, None] == np.arange(P)[None, None, :]).astype(oh_dt)
        per_core.append({"gidx": np.ascontiguousarray(gidx),
                         "ewt": np.ascontiguousarray(ewt_t),
                         "oh": np.ascontiguousarray(oh)})
    return per_core, schedule, ntiles


def _shardT(x, c):
    dt = np.float16 if NODE16 else np.float32
    sh = np.zeros((x.shape[1], NDP), dt)
    sh[:, :ND] = x[c * ND:(c + 1) * ND].T.astype(dt)
    return sh


# ---------------------------------------------------------------- device IR

def _build_program(schedules, ntiles_v2c, ntiles_c2v, no_collective=False,
                   no_gather=False, act_dve=False):
    import concourse.bacc as bacc
    import concourse.mybir as mybir
    import concourse.tile as tile
    from concourse.masks import make_identity

    f32, f16, i16 = mybir.dt.float32, mybir.dt.float16, mybir.dt.int16
    f8 = mybir.dt.float8e4 if OHDT == "f8" else mybir.dt.float16
    fe = f16 if EDGE16 else f32   # edge pipeline dtype
    fn = f16 if NODE16 else f32   # node feature / MLP dtype
    AF = mybir.ActivationFunctionType
    OP = mybir.AluOpType

    nc = bacc.Bacc("TRN2", target_bir_lowering=False, debug=False,
                   num_devices=NCORES, num_swdge_queues=NQ,
                   dynamic_dma_scratch_size=32768)

    # ---------------- dram tensor declarations
    def din(name, shape, dt=f32):
        return nc.dram_tensor(name, shape, dt, kind="ExternalInput")

    ecv, ecc = ntiles_v2c * P, ntiles_c2v * P
    dirs = {}
    for d, ec in (("v2c", ecv), ("c2v", ecc)):
        dirs[d] = {
            "gidx": din(f"{d}_gidx", [P, ec // 16], i16),
            "ewt": din(f"{d}_ewt", [P, ec // P], fe),
            "oh": din(f"{d}_oh", [P, ec // P, P], f8),
            "lew": din(f"{d}_lew", [P, NL * HID], fe),
            "w1": din(f"{d}_w1", [NL, HID, 2 * HID], fn),
            "w2": din(f"{d}_w2", [NL, 2 * HID, HID], fn),
            "b1": din(f"{d}_b1", [2 * HID, NL]),
            "b2": din(f"{d}_b2", [HID, NL]),
            "b2leb": din(f"{d}_b2leb", [P, NL * HID], f16),
            "leb": din(f"{d}_leb", [HID, NL]),
        }
    enc = {}
    for t in ("vals", "cons"):
        enc[t] = {
            "xT": din(f"{t}_xT", [2, NDP], fn),
            "peT": din(f"{t}_peT", [8, NDP], fn),
            "peTn": din(f"{t}_peTn", [8, NDP], fn),
            "ew": din(f"{t}_enc_w", [2, HID // 2], fn),
            "eb": din(f"{t}_enc_b", [HID // 2, 1]),
            "pw1": din(f"{t}_pe_w1", [8, HID], fn),
            "pb1": din(f"{t}_pe_b1", [HID, 1]),
            "pw2": din(f"{t}_pe_w2", [HID, HID // 2], fn),
            "pb2": din(f"{t}_pe_b2", [HID // 2, 1]),
            "prw1": din(f"{t}_pred_w1", [HID, HID], fn),
            "prb1": din(f"{t}_pred_b1", [HID, 1]),
            "prw2": din(f"{t}_pred_w2", [HID, 1], fn),
            "prb2": din(f"{t}_pred_b2", [1, 1]),
        }
    pv_out = nc.dram_tensor("pv_out", [NL, NDP], f32, kind="ExternalOutput")
    pc_out = nc.dram_tensor("pc_out", [NL, NDP], f32, kind="ExternalOutput")

    # per-exchange internal tensors: 6 tables (XV0, XC1, XV1, XC2, XV2, XC3)
    tables = []
    cc_ins = []
    tablecs = []
    ftbl = f16 if TBLDT == "f16" else f32
    TW = P if TBLDT == "f16" else HID  # table row width (elements)
    for k in range(6):
        cc_ins.append(nc.dram_tensor(f"cc_in_{k}", [NDP, HID], ftbl, kind="Internal"))
        if TBLDT == "f16":
            tablecs.append(nc.dram_tensor(f"tablec_{k}", [TBL, HID], f16,
                                          kind="Internal", addr_space="Shared"))
            tables.append(nc.dram_tensor(f"table_{k}", [TBL, P], f16,
                                         kind="Internal"))
        else:
            tables.append(nc.dram_tensor(f"table_{k}", [TBL, HID], f32,
                                         kind="Internal", addr_space="Shared"))
            tablecs.append(tables[-1])

    RG = [list(range(NCORES))]

    with tile.TileContext(nc) as tc:
        from contextlib import ExitStack
        with ExitStack() as ctx:
            const = ctx.enter_context(tc.tile_pool(name="const", bufs=1))
            nodes = ctx.enter_context(tc.tile_pool(name="nodes", bufs=1))
            pseg = ctx.enter_context(tc.tile_pool(name="pseg", bufs=5, space="PSUM"))
            pmlp = ctx.enter_context(tc.tile_pool(name="pmlp", bufs=1, space="PSUM"))
            pmlp2 = ctx.enter_context(tc.tile_pool(name="pmlp2", bufs=1, space="PSUM"))
            ptr = ctx.enter_context(tc.tile_pool(name="ptr", bufs=1, space="PSUM"))

            def load_const(pool, dram, shape, dt=f32, tag=None, in_ap=None,
                           out_3d=None):
                t = pool.tile(shape, dt, tag=tag or dram.name, name="lc")
                out_ap = t[:] if out_3d is None else t[:].rearrange(
                    "k (l m) -> k l m", l=out_3d)
                nc.sync.dma_start(out=out_ap,
                                  in_=in_ap if in_ap is not None else dram[:])
                return t

            ident = const.tile([P, P], f16 if TBLDT == "f16" else f32,
                               tag="ident")
            make_identity(nc, ident[:])
            epsb = const.tile([HID, 1], f32, tag="epsb")
            nc.vector.memset(epsb[:], 1e-16)

            dsb = {}
            for d in ("v2c", "c2v"):
                dd = dirs[d]
                ec = ecv if d == "v2c" else ecc
                dsb[d] = {
                    "gidx": load_const(const, dd["gidx"], [P, ec // 16], i16),
                    "ewt": load_const(const, dd["ewt"], [P, ec // P], fe),
                    "lew": load_const(const, dd["lew"], [P, NL * HID], fe),
                    "w1": load_const(const, dd["w1"], [HID, NL * 2 * HID], fn,
                                     out_3d=NL,
                                     in_ap=dd["w1"][:].rearrange("l k m -> k l m")),
                    "w2": load_const(const, dd["w2"], [2 * HID, NL * HID], fn,
                                     out_3d=NL,
                                     in_ap=dd["w2"][:].rearrange("l k m -> k l m")),
                    "b1": load_const(const, dd["b1"], [2 * HID, NL]),
                    "b2": load_const(const, dd["b2"], [HID, NL]),
                    "b2leb": load_const(const, dd["b2leb"], [P, NL * HID], f16),
                    "leb": load_const(const, dd["leb"], [HID, NL]),
                    "oh_dram": dd["oh"],
                    "ntiles": ec // P,
                }
            esb = {}
            _f16keys = ("ew", "pw1", "pw2", "prw1", "prw2")
            for t in ("vals", "cons"):
                ee = enc[t]
                esb[t] = {k: load_const(const, ee[k], list(ee[k].shape),
                                        fn if k in _f16keys else f32,
                                        tag=f"{t}_{k}")
                          for k in ("ew", "eb", "pw1", "pb1", "pw2", "pb2",
                                    "prw1", "prb1", "prw2", "prb2")}

            NCHUNK = NDP // 512  # 5

            xv_ab = [nodes.tile([HID, NDP], fn, tag="xv_a", name="xv_a"),
                     nodes.tile([HID, NDP], fn, tag="xv_b", name="xv_b")]
            xc_ab = [nodes.tile([HID, NDP], fn, tag="xc_a", name="xc_a"),
                     nodes.tile([HID, NDP], fn, tag="xc_b", name="xc_b")]

            # ---------------- encoder (scoped input pool, freed afterwards)
            with tc.tile_pool(name="encio", bufs=1) as encio:
                eio = {}
                for t in ("vals", "cons"):
                    ee = enc[t]
                    eio[t] = {k: load_const(encio, ee[k], list(ee[k].shape),
                                            fn, tag=f"{t}_{k}")
                              for k in ("xT", "peT", "peTn")}

                def encoder(t, out_tile):
                    e = esb[t]
                    io = eio[t]
                    for ch in range(NCHUNK):
                        sl = slice(ch * 512, (ch + 1) * 512)
                        pm = pmlp.tile([HID // 2, 512], f32, tag="pm1", name="pm")
                        nc.tensor.matmul(pm[:], lhsT=e["ew"][:], rhs=io["xT"][:, sl],
                                         start=True, stop=True)
                        nc.scalar.activation(out_tile[0:HID // 2, sl], pm[:],
                                             AF.Relu, bias=e["eb"][:])
                        hpe = encio.tile([HID, 512], fn, tag="hpe", name="hpe")
                        pp = pmlp2.tile([HID, 512], f32, tag="pm2", name="pp")
                        nc.tensor.matmul(pp[:], lhsT=e["pw1"][:], rhs=io["peT"][:, sl],
                                         start=True, stop=True)
                        nc.scalar.activation(hpe[:], pp[:], AF.Relu, bias=e["pb1"][:])
                        hpen = encio.tile([HID, 512], fn, tag="hpen", name="hpen")
                        ppn = pmlp2.tile([HID, 512], f32, tag="pm2", name="ppn")
                        nc.tensor.matmul(ppn[:], lhsT=e["pw1"][:],
                                         rhs=io["peTn"][:, sl],
                                         start=True, stop=True)
                        nc.scalar.activation(hpen[:], ppn[:], AF.Relu,
                                             bias=e["pb1"][:])
                        p2 = pmlp.tile([HID // 2, 512], f32, tag="pm1", name="p2e")
                        nc.tensor.matmul(p2[:], lhsT=e["pw2"][:], rhs=hpe[:],
                                         start=True, stop=False)
                        nc.tensor.matmul(p2[:], lhsT=e["pw2"][:], rhs=hpen[:],
                                         start=False, stop=True)
                        nc.scalar.activation(out_tile[HID // 2:HID, sl], p2[:],
                                             AF.Relu, bias=e["pb2"][:], scale=0.5)

                encoder("vals", xv_ab[0])
                encoder("cons", xc_ab[0])

            # ---------------- main pools
            stream = ctx.enter_context(tc.tile_pool(name="stream", bufs=SBUFS))
            work = ctx.enter_context(tc.tile_pool(name="work", bufs=WBUFS))
            wt = ctx.enter_context(tc.tile_pool(name="wt", bufs=1))
            outpre = nodes.tile([HID, NDP], fn, tag="outpre", name="outpre")
            nmt = nodes.tile([P, NSEG * HID], f16, tag="nmt", name="nmt")

            # ---------------- table write + exchange
            def write_table(src_tile, leb_col, k, nm_pre=False):
                """Node-major payload (prebuilt in nmt by conv_layer, or via
                transposes for the encoder) -> cc_in_k -> AllGather."""
                ftw = f16 if TBLDT == "f16" else f32
                if nm_pre:
                    nm = nmt
                else:
                    tleb = wt.tile([HID, NDP], ftw, tag="tleb", name="tleb")
                    nc.scalar.activation(tleb[:], src_tile[:], AF.Identity,
                                         bias=leb_col)
                    nm = wt.tile([P, NSEG * HID], ftw, tag="nm", name="nm")
                    for s in range(NSEG):
                        pt = ptr.tile([P, HID], ftw, tag="pt", name="pt")
                        nc.tensor.transpose(pt[:], tleb[:, s * P:(s + 1) * P],
                                            ident[:HID, :HID])
                        nc.vector.tensor_copy(nm[:, s * HID:(s + 1) * HID],
                                              pt[:])
                nc.sync.dma_start(
                    out=cc_ins[k][:].rearrange("(s p) f -> p s f", p=P),
                    in_=nm[:].rearrange("p (s f) -> p s f", f=HID))
                if no_collective:
                    nc.sync.dma_start(out=tables[k][0:NDP, 0:HID], in_=cc_ins[k][:])
                else:
                    nc.gpsimd.collective_compute(
                        "AllGather", OP.bypass,
                        ins=[cc_ins[k][:]], outs=[tablecs[k][:]],
                        replica_groups=RG)
                    if TBLDT == "f16":
                        nc.sync.dma_start(out=tables[k][:, 0:HID],
                                          in_=tablecs[k][:])

            # ---------------- one message-passing layer
            def conv_layer(d, i, x_dst, out_tile, table_in, make_nm=True):
                sb = dsb[d]
                sched = schedules[d]
                ntl = sb["ntiles"]
                ngroups = ntl // TPB
                lew_b = sb["lew"][:, i * HID:(i + 1) * HID].unsqueeze(1) \
                    .to_broadcast([P, TPB, HID])
                segpsum = {}
                for g in range(ngroups):
                    gtw = P if TBLDT == "f16" else HID
                    gt = stream.tile([P, TPB * gtw], f16 if TBLDT == "f16" else f32,
                                     tag="gather", name="gt")
                    gt3 = gt[:].rearrange("p (t f) -> p t f", f=gtw)
                    if no_gather:
                        nc.sync.dma_start(
                            out=gt3,
                            in_=table_in[0:GROUP, :].rearrange(
                                "(t p) f -> p t f", p=P))
                    else:
                        nc.gpsimd.dma_gather(
                            gt3, table_in[:],
                            sb["gidx"][:, g * (GROUP // 16):(g + 1) * (GROUP // 16)],
                            num_idxs=GROUP, num_idxs_reg=GROUP, elem_size=gtw,
                            single_packet=False, queue_num=g % NQ)
                    oh = stream.tile([P, TPB * P], f8, tag="oh", name="oh")
                    nc.sync.dma_start(out=oh[:],
                                      in_=sb["oh_dram"][:, g * TPB:(g + 1) * TPB, :])
                    ewt_b = sb["ewt"][:, g * TPB:(g + 1) * TPB].to_broadcast(
                        [P, TPB, HID])
                    cm = work.tile([P, TPB * HID], fe, tag="cm", name="cm")
                    cm3 = cm[:].rearrange("p (t f) -> p t f", f=HID)
                    nc.vector.tensor_tensor(out=cm3, in0=ewt_b, in1=lew_b,
                                            op=OP.mult)
                    m0 = work.tile([P, TPB * HID], fe, tag="m0", name="m0")
                    m3 = m0[:].rearrange("p (t f) -> p t f", f=HID)
                    nc.vector.tensor_tensor(out=m3, in0=cm3,
                                            in1=gt3[:, :, 0:HID], op=OP.add)

                    r16 = work.tile([P, TPB * HID], f16, tag="r16", name="r16")
                    nc.scalar.activation(r16[:], m0[:], AF.Relu)
                    v16 = stream.tile([P, TPB * P], f16, tag="v16", name="v16")
                    v3 = v16[:].rearrange("p (t f) -> p t f", f=P)
                    r3 = r16[:].rearrange("p (t f) -> p t f", f=HID)
                    nc.scalar.activation(v3[:, :, 0:HID], r3, AF.Exp)
                    nc.vector.tensor_tensor(out=v3[:, :, HID:P],
                                            in0=v3[:, :, 0:HID], in1=r3,
                                            op=OP.mult)
                    oh3 = oh[:].rearrange("p (t f) -> p t f", f=P)
                    for t in range(TPB):
                        gt_i = g * TPB + t
                        st, is_start, is_stop = sched[gt_i]
                        if is_start:
                            segpsum[st] = pseg.tile([P, P], f32, tag="seg",
                                                    name="segps")
                        nc.tensor.matmul(segpsum[st][:],
                                         lhsT=v3[:, t, :], rhs=oh3[:, t, :],
                                         start=is_start, stop=is_stop)
                        if is_stop:
                            ps = segpsum.pop(st)
                            sl = slice(st * P, (st + 1) * P)
                            sg = work.tile([HID, P], f32, tag="sg", name="sg")
                            nc.scalar.activation(sg[:], ps[0:HID, :],
                                                 AF.Identity, bias=epsb[:])
                            rec = work.tile([HID, P], f32, tag="rec", name="rec")
                            nc.vector.reciprocal(rec[:], sg[:])
                            nc.vector.tensor_tensor(out=outpre[:, sl],
                                                    in0=ps[HID:P, :],
                                                    in1=rec[:], op=OP.mult)
                # MLP: out = W2^T relu(W1^T outpre + b1) + b2.  Node-major
                # table rows come from extra [h_chunk^T x W2] matmuls (no PE
                # transposes -> the AllGather is not serialized against them).
                w1 = sb["w1"][:, i * 2 * HID:(i + 1) * 2 * HID]
                w2 = sb["w2"][:, i * HID:(i + 1) * HID]
                b2leb = sb["b2leb"][:, i * HID:(i + 1) * HID]
                for ch in range(NCHUNK):
                    sl = slice(ch * 512, (ch + 1) * 512)
                    p1 = pmlp.tile([2 * HID, 512], f32, tag="pm1", name="p1")
                    nc.tensor.matmul(p1[:], lhsT=w1, rhs=outpre[:, sl],
                                     start=True, stop=False)
                    nc.tensor.matmul(p1[:], lhsT=w1, rhs=x_dst[:, sl],
                                     start=False, stop=True)
                    h = work.tile([2 * HID, 512], fn, tag="h", name="h")
                    if act_dve:
                        nc.vector.tensor_scalar(out=h[:], in0=p1[:],
                                                scalar1=sb["b1"][:, i:i + 1],
                                                scalar2=0.0, op0=OP.add, op1=OP.max)
                    else:
                        nc.scalar.activation(h[:], p1[:], AF.Relu,
                                             bias=sb["b1"][:, i:i + 1])
                    p2 = pmlp2.tile([HID, 512], f32, tag="pm2", name="p2")
                    nc.tensor.matmul(p2[:], lhsT=w2, rhs=h[:],
                                     start=True, stop=True)
                    nc.scalar.activation(out_tile[:, sl], p2[:], AF.Identity,
                                         bias=sb["b2"][:, i:i + 1])
                    if make_nm:
                        for q in range(4):
                            sseg = ch * 4 + q
                            pn = ptr.tile([P, HID], f32, tag="pt", name="pn")
                            nc.tensor.matmul(
                                pn[:], lhsT=h[:, q * P:(q + 1) * P],
                                rhs=w2, start=True, stop=True)
                            nc.vector.tensor_tensor(
                                out=nmt[:, sseg * HID:(sseg + 1) * HID],
                                in0=pn[:], in1=b2leb, op=OP.add)

            # ---------------- prediction head (inline per layer)
            def pred_head(t, i, h_tile, out_dram):
                e = esb[t]
                for ch in range(NCHUNK):
                    sl = slice(ch * 512, (ch + 1) * 512)
                    p1 = pmlp.tile([HID, 512], f32, tag="pm1", name="pp1")
                    nc.tensor.matmul(p1[:], lhsT=e["prw1"][:], rhs=h_tile[:, sl],
                                     start=True, stop=True)
                    ph = work.tile([2 * HID, 512], fn, tag="h", name="ph")
                    if act_dve:
                        nc.vector.tensor_scalar(out=ph[:HID, :], in0=p1[:],
                                                scalar1=e["prb1"][:],
                                                scalar2=0.0, op0=OP.add, op1=OP.max)
                    else:
                        nc.scalar.activation(ph[:HID, :], p1[:], AF.Relu,
                                             bias=e["prb1"][:])
                    p2 = pmlp2.tile([1, 512], f32, tag="pm2", name="pp2")
                    nc.tensor.matmul(p2[:], lhsT=e["prw2"][:], rhs=ph[:HID, :],
                                     start=True, stop=True)
                    po = work.tile([1, 512], f32, tag="po", name="po")
                    nc.vector.tensor_scalar(out=po[:], in0=p2[:],
                                            scalar1=e["prb2"][:],
                                            scalar2=None, op0=OP.add)
                    nc.sync.dma_start(out=out_dram[i:i + 1, sl], in_=po[:])

            # ---------------- main sequence
            # exchange k: 0=XV0, 1=XC1, 2=XV1, 3=XC2, 4=XV2, 5=XC3
            write_table(xv_ab[0], dsb["v2c"]["leb"][:, 0:1], 0)
            for i in range(NL):
                new_xc = xc_ab[(i + 1) % 2]
                conv_layer("v2c", i, xc_ab[i % 2], new_xc, tables[2 * i])
                k = 2 * i + 1
                write_table(None, None, k, nm_pre=True)
                pred_head("cons", i, new_xc, pc_out)
                new_xv = xv_ab[(i + 1) % 2]
                conv_layer("c2v", i, xv_ab[i % 2], new_xv, tables[k],
                           make_nm=(i < NL - 1))
                if i < NL - 1:
                    write_table(None, None, 2 * (i + 1), nm_pre=True)
                pred_head("vals", i, new_xv, pv_out)

    nc.compile()
    return nc


# ---------------------------------------------------------------- entry

def kernel(**inputs):
    from concourse.bass_utils import run_bass_kernel_spmd

    inp = {k: np.asarray(v) for k, v in inputs.items()}

    v2c_cores, v2c_sched, ntv = _prep_direction(
        inp["edge_index_v2c"][0], inp["edge_index_v2c"][1], inp["edge_weight_v2c"])
    c2v_cores, c2v_sched, ntc = _prep_direction(
        inp["edge_index_c2v"][0], inp["edge_index_c2v"][1], inp["edge_weight_c2v"])

    import os as _os
    _nocc = bool(_os.environ.get("GNN_NOCC"))
    _nogather = bool(_os.environ.get("GNN_NOGATHER"))
    _actdve = _os.environ.get("GNN_ACTDVE", "0") != "0"
    key = (ntv, ntc, _nocc, _nogather, _actdve, TPB, NQ, SBUFS, WBUFS, OHDT, TBLDT, EDGE16, NODE16,
           tuple(s[0] for s in v2c_sched), tuple(s[0] for s in c2v_sched))
    if key not in _PROG_CACHE:
        _PROG_CACHE[key] = _build_program(
            {"v2c": v2c_sched, "c2v": c2v_sched}, ntv, ntc,
            no_collective=_nocc, no_gather=_nogather, act_dve=_actdve)
    nc = _PROG_CACHE[key]

    # ---- shared (replicated) weight tensors
    shared = {}
    for d in ("v2c", "c2v"):
        lew = inp[f"{d}_edge_w"][:, 0, :]            # [NL, HID]
        shared[f"{d}_lew"] = np.tile(lew.reshape(1, NL * HID), (P, 1)).astype(
            np.float16 if EDGE16 else np.float32)
        w1 = inp[f"{d}_w1"].astype(np.float32)       # [NL, HID, 2H]
        b1 = inp[f"{d}_b1"].astype(np.float32)       # [NL, 2H]
        # fold msg eps: out_pre_true = out_pre + EPS (per feature, all features)
        # -> b1' = b1 + EPS * sum_f w1[f, :]
        b1p = b1 + EPS * w1.sum(axis=1)
        _ndt = np.float16 if NODE16 else np.float32
        shared[f"{d}_w1"] = w1.astype(_ndt)
        shared[f"{d}_w2"] = inp[f"{d}_w2"].astype(_ndt)
        shared[f"{d}_b1"] = np.ascontiguousarray(b1p.T)          # [2H, NL]
        shared[f"{d}_b2"] = np.ascontiguousarray(inp[f"{d}_b2"].T)  # [H, NL]
        shared[f"{d}_leb"] = np.ascontiguousarray(inp[f"{d}_edge_b"].T)  # [H, NL]
    # folded (own b2 + consumer-layer edge bias) broadcast rows, node-major
    v2c_b2 = inp["v2c_b2"].astype(np.float32)   # [NL, H]
    c2v_b2 = inp["c2v_b2"].astype(np.float32)
    v2c_leb = inp["v2c_edge_b"].astype(np.float32)  # [NL, H]
    c2v_leb = inp["c2v_edge_b"].astype(np.float32)
    bl_v = np.zeros((NL, HID), np.float32)  # v2c conv i -> XC table k=2i+1
    bl_c = np.zeros((NL, HID), np.float32)  # c2v conv i -> XV table k=2i+2
    for i in range(NL):
        bl_v[i] = v2c_b2[i] + c2v_leb[i]
        if i < NL - 1:
            bl_c[i] = c2v_b2[i] + v2c_leb[i + 1]
    shared["v2c_b2leb"] = np.tile(bl_v.reshape(1, NL * HID),
                                  (P, 1)).astype(np.float16)
    shared["c2v_b2leb"] = np.tile(bl_c.reshape(1, NL * HID),
                                  (P, 1)).astype(np.float16)
    for t in ("vals", "cons"):
        _ndt = np.float16 if NODE16 else np.float32
        shared[f"{t}_enc_w"] = inp[f"enc_{t}_w"].astype(_ndt)
        shared[f"{t}_enc_b"] = inp[f"enc_{t}_b"].reshape(-1, 1).astype(np.float32)
        shared[f"{t}_pe_w1"] = inp[f"pe_{t}_w1"].astype(_ndt)
        shared[f"{t}_pe_b1"] = inp[f"pe_{t}_b1"].reshape(-1, 1).astype(np.float32)
        shared[f"{t}_pe_w2"] = inp[f"pe_{t}_w2"].astype(_ndt)
        shared[f"{t}_pe_b2"] = inp[f"pe_{t}_b2"].reshape(-1, 1).astype(np.float32)
        shared[f"{t}_pred_w1"] = inp[f"pred_{t}_w1"].astype(_ndt)
        shared[f"{t}_pred_b1"] = inp[f"pred_{t}_b1"].reshape(-1, 1).astype(np.float32)
        shared[f"{t}_pred_w2"] = inp[f"pred_{t}_w2"].astype(_ndt)
        shared[f"{t}_pred_b2"] = inp[f"pred_{t}_b2"].reshape(-1, 1).astype(np.float32)

    in_maps = []
    for c in range(NCORES):
        m = dict(shared)
        for d, cores in (("v2c", v2c_cores), ("c2v", c2v_cores)):
            m[f"{d}_gidx"] = cores[c]["gidx"]
            m[f"{d}_ewt"] = cores[c]["ewt"]
            m[f"{d}_oh"] = cores[c]["oh"]
        for t, x, pe in (("vals", inp["x_vals"], inp["pe_vals"]),
                         ("cons", inp["x_cons"], inp["pe_cons"])):
            m[f"{t}_xT"] = _shardT(x, c)
            peT = _shardT(pe, c)
            m[f"{t}_peT"] = peT
            m[f"{t}_peTn"] = -peT
        in_maps.append(m)

    import os
    global LAST_EXEC_NS
    nbench = int(os.environ.get("GNN_BENCH", "0"))
    if nbench:
        results, LAST_EXEC_NS = _run_benched(nc, in_maps, nbench)
    elif os.environ.get("GNN_SIM"):
        from concourse.bass_interp import MultiCoreSim
        sim = MultiCoreSim(nc, num_cores=NCORES, num_workers=8)
        for c, cs in sim.cores.items():
            for k, v in in_maps[c].items():
                cs.tensor(k)[:] = v
        sim.simulate(check_with_hw=False)
        results = [{k: np.asarray(sim.cores[c].tensor(k))
                    for k in ("pv_out", "pc_out")} for c in range(NCORES)]
    else:
        res = run_bass_kernel_spmd(nc, in_maps, core_ids=list(range(NCORES)))
        LAST_EXEC_NS = res.exec_time_ns
        results = res.results

    pv = np.zeros((NV, NL), np.float32)
    pc = np.zeros((NC, NL), np.float32)
    for c in range(NCORES):
        pv[c * ND:(c + 1) * ND] = results[c]["pv_out"][:, :ND].T
        pc[c * ND:(c + 1) * ND] = results[c]["pc_out"][:, :ND].T
    return pv, pc


LAST_EXEC_NS = None
LAST_FLOOR_NS = None
_FLOOR_PROG = None


def _floor_prog():
    """Trivial 8-core program used to measure the per-dispatch overhead."""
    global _FLOOR_PROG
    if _FLOOR_PROG is None:
        import concourse.bacc as bacc
        import concourse.mybir as mybir
        import concourse.tile as tile
        f32 = mybir.dt.float32
        fnc = bacc.Bacc("TRN2", target_bir_lowering=False, debug=False,
                        num_devices=NCORES)
        xin = fnc.dram_tensor("xin", [P, P], f32, kind="ExternalInput")
        xout = fnc.dram_tensor("xout", [P, P], f32, kind="ExternalOutput")
        with tile.TileContext(fnc) as tc:
            with tc.tile_pool(name="p", bufs=1) as pool:
                t = pool.tile([P, P], f32)
                fnc.sync.dma_start(out=t[:], in_=xin[:])
                fnc.sync.dma_start(out=xout[:], in_=t[:])
        fnc.compile()
        _FLOOR_PROG = fnc
    return _FLOOR_PROG


def _run_benched(nc, in_maps, niter):
    """Bench the main program, then the trivial floor program (dispatch
    overhead calibration). Prints both; returns the raw main-loop time."""
    import os
    global LAST_FLOOR_NS
    results, ns = _bench_once(nc, in_maps, niter)
    if os.environ.get("GNN_FLOOR", "1") != "0":
        fnc = _floor_prog()
        fmaps = [{"xin": np.zeros((P, P), np.float32)} for _ in range(NCORES)]
        _, fns = _bench_once(fnc, fmaps, niter)
        LAST_FLOOR_NS = fns
        print(f"[bench] floor: {fns/1e6:.3f} ms/iter; "
              f"kernel-only: {(ns - fns)/1e6:.3f} ms/iter")
    return results, ns


def _bench_once(nc, in_maps, niter):
    """Compile once via the bass2jax PJRT path, then time `niter` executions
    with device-resident inputs. Returns (results, per-iter exec ns)."""
    import time
    import jax
    import jax.numpy as jnp
    from jax.sharding import Mesh, PartitionSpec
    from jax.experimental.shard_map import shard_map
    import concourse.mybir as mybir
    from concourse import bass2jax

    bass2jax.install_neuronx_cc_hook()
    partition_name = nc.partition_id_tensor.name if nc.partition_id_tensor else None
    in_names, out_names, out_avals = [], [], []
    for alloc in nc.m.functions[0].allocations:
        if not isinstance(alloc, mybir.MemoryLocationSet):
            continue
        name = alloc.memorylocations[0].name
        if alloc.kind == "ExternalInput":
            if name != partition_name:
                in_names.append(name)
        elif alloc.kind == "ExternalOutput":
            out_names.append(name)
            out_avals.append(jax.core.ShapedArray(
                tuple(alloc.tensor_shape), mybir.dt.np(alloc.dtype)))
    n_params = len(in_names)
    all_in_names = in_names + out_names
    if partition_name is not None:
        all_in_names = all_in_names + [partition_name]

    import jax.numpy as _jnp

    def _call_once(ins, zeros_ops):
        operands = list(ins) + list(zeros_ops)
        if partition_name is not None:
            operands.append(bass2jax.partition_id_tensor())
        outs = bass2jax._bass_exec_p.bind(
            *operands,
            out_avals=tuple(out_avals),
            in_names=tuple(all_in_names),
            out_names=tuple(out_names),
            lowering_input_output_aliases=(),
            sim_require_finite=True,
            sim_require_nnan=True,
            nc=nc,
        )
        return tuple(outs)

    def _make_body(nloop):
        def _body(*args):
            ins = args[:n_params]
            zeros_ops = args[n_params:]
            outs = _call_once(ins, zeros_ops)
            for _ in range(nloop - 1):
                zeros_ops = tuple(o * 0 for o in outs)
                outs = _call_once(ins, zeros_ops)
            return outs
        return _body
    _body = _make_body(1)

    devices = jax.devices()[:NCORES]
    mesh = Mesh(np.asarray(devices), ("core",))
    n_outs = len(out_names)
    in_specs = (PartitionSpec("core"),) * (n_params + n_outs)
    out_specs = (PartitionSpec("core"),) * n_outs
    def make_sharded(nloop):
        return jax.jit(
            shard_map(_make_body(nloop), mesh=mesh, in_specs=in_specs,
                      out_specs=out_specs, check_rep=False),
            donate_argnums=tuple(range(n_params, n_params + n_outs)),
            keep_unused=True)
    sharded = make_sharded(1)

    from jax.sharding import NamedSharding
    shard = NamedSharding(mesh, PartitionSpec("core"))
    dev_in = []
    for i, name in enumerate(in_names):
        cat = np.concatenate([np.asarray(in_maps[c][name]) for c in range(NCORES)],
                             axis=0)
        dev_in.append(jax.device_put(cat, shard))

    def zeros():
        return [jax.device_put(
            np.zeros((NCORES * a.shape[0], *a.shape[1:]), a.dtype), shard)
            for a in out_avals]

    # warmup (compiles)
    out = sharded(*dev_in, *zeros())
    jax.block_until_ready(out)

    def timed(fn, reps=3):
        best = float("inf")
        for _ in range(reps):
            z = zeros()
            jax.block_until_ready(z)
            t0 = time.perf_counter()
            o = fn(*dev_in, *z)
            jax.block_until_ready(o)
            best = min(best, time.perf_counter() - t0)
        return best

    # async sequential loop: per-iter amortized time (dispatch latency pipelines)
    zs = [zeros() for _ in range(niter)]
    jax.block_until_ready(zs)
    t0 = time.perf_counter()
    outs = None
    for k in range(niter):
        outs = sharded(*dev_in, *zs[k])
    jax.block_until_ready(outs)
    dt = (time.perf_counter() - t0) / niter
    print(f"[bench] async loop x{niter}: {dt*1e3:.3f} ms/iter")
    exec_ns = int(dt * 1e9)
    out = outs
    results = []
    for c in range(NCORES):
        results.append({
            name: np.asarray(out[i]).reshape(NCORES, *out_avals[i].shape)[c]
            for i, name in enumerate(out_names)})
    return results, exec_ns



# revision 58
# speedup vs baseline: 2.1562x; 2.1562x over previous
"""Trainium2 Bass kernel for BipartiteHeteroGNN (gnn_message_passing).

Strategy (8 NeuronCores, SPMD):
- Nodes (vals/cons) sharded by id: core c owns ids [c*2500, (c+1)*2500).
- Edges assigned by destination core, sorted by dst, grouped into 128-dst
  "segment tiles"; per-edge src features fetched with dma_gather (256B rows,
  64 fp16 features + 64 pad) from a replicated node table in HBM, using
  4 SWDGE queues and small (TPB=8) groups for deep pipelining.
- fp16 end to end: node features, edge-message pipeline, MLP/encoder/pred
  weights, gather tables and AllGather payloads (fp32 only in PSUM and
  biases); rel-err stays ~5e-4, well under the 2e-2 gate.
- Segment softmax without segment-max (messages are relu(..)+eps >= 0 and
  bounded, so exp() never overflows; guard 1e-16 keeps empty segments at 0).
- Scatter-add per segment tile via one-hot matmul on the tensor engine
  (fp16 one-hots precomputed on host; fp16 ex/p values; fp32 PSUM accum).
- Node MLPs in feature-major ("T") layout so biases are per-partition;
  MLP relu / bias adds / transpose copies run on the Activation engine to
  unload the Vector engine (the busiest compute engine in this kernel).
- Cross-core exchange of updated fp16 node features via AllGather collective
  into a compact [TBL, 64] staging table, then one strided DMA re-pads rows
  to 256B for the gather (+edge-bias of the consumer layer pre-folded).

Bench notes: the harness's timed loop carries a ~2.5-3.5 ms/iter axon
dispatch floor that drifts with machine load; _run_benched also times a
trivial kernel to report a floor-calibrated kernel-only number.
"""
import numpy as np

P = 128
NCORES = 8
NV = NC = 20000
E = 500000
HID = 64
NL = 3
EPS = 1e-7
ND = 2500          # dst nodes per core (per node type)
NSEG = 20          # segment tiles per core (ceil(2500/128))
NDP = NSEG * P     # padded dst nodes per core = 2560
TBL = NCORES * NDP # gather table rows = 20480
import os as _os0
TPB = int(_os0.environ.get("GNN_TPB", "8"))   # edge tiles per gather group
GROUP = TPB * P    # edges per gather group
NQ = int(_os0.environ.get("GNN_QUEUES", "4"))  # SWDGE queues (max 4)
SBUFS = int(_os0.environ.get("GNN_SBUFS", "6"))  # stream pool bufs
WBUFS = int(_os0.environ.get("GNN_WBUFS", "4"))  # work pool bufs
PSEGB = int(_os0.environ.get("GNN_PSEG", "5"))   # segment PSUM bufs
PMLPB = int(_os0.environ.get("GNN_PMLP", "1"))   # MLP psum bufs
PMLP2B = int(_os0.environ.get("GNN_PMLP2", "1"))
PTRB = int(_os0.environ.get("GNN_PTR", "1"))     # nm-transpose psum bufs
SPK = _os0.environ.get("GNN_SPK", "0") != "0"    # dma_gather single_packet
SRCSORT = _os0.environ.get("GNN_SRCSORT", "1") != "0"  # sort segtile edges by src
OHDT = _os0.environ.get("GNN_OHDT", "f8")        # one-hot dtype: f8 | f16
OHRES = _os0.environ.get("GNN_OHRES", "0") != "0"  # v2c one-hots SBUF-resident
OHRESG = int(_os0.environ.get("GNN_OHRESG", "52"))  # resident v2c groups
TBLDT = _os0.environ.get("GNN_TBLDT", "f16")      # gather table dtype: f16 | f32
EDGE16 = _os0.environ.get("GNN_EDGE16", "1") != "0"  # f16 edge msg pipeline
NODE16 = _os0.environ.get("GNN_NODE16", "1") != "0"  # f16 node feats + MLPs
F32 = None         # set lazily (mybir)
F16 = None
I16 = None

_PROG_CACHE = {}


# ---------------------------------------------------------------- host prep

def _prep_direction(src, dst, ewt):
    """Edge preprocessing for one direction.

    Returns (per_core, schedule, ntiles):
      per_core[c] = dict(gidx [128, EC//16] i16, ewt [128, EC//128] f32,
                         oh [128, EC//128, 128] f16)
      schedule[t] = (segtile, is_start, is_stop) for each edge tile t.
    """
    src = np.asarray(src)
    dst = np.asarray(dst)
    ewt = np.asarray(ewt).reshape(-1)
    cores = []
    counts = np.zeros((NCORES, NSEG), np.int64)
    for c in range(NCORES):
        lo = c * ND
        m = (dst >= lo) & (dst < lo + ND)
        s_c = src[m]
        d_c = (dst[m] - lo).astype(np.int64)
        w_c = ewt[m]
        order = np.argsort(d_c, kind="stable")
        s_c, d_c, w_c = s_c[order], d_c[order], w_c[order]
        st_of = d_c // P
        bounds = np.searchsorted(st_of, np.arange(NSEG + 1))
        cores.append((s_c, d_c, w_c, bounds))
        counts[c] = bounds[1:] - bounds[:-1]
    st_tiles = np.maximum(1, np.ceil(counts.max(axis=0) / P).astype(np.int64))
    ntiles = int(st_tiles.sum())
    pad_tiles = (-ntiles) % TPB
    st_tiles[NSEG - 1] += pad_tiles       # merge trailing pads into last segtile
    ntiles += pad_tiles
    EC = ntiles * P

    schedule = []
    for st in range(NSEG):
        for k in range(st_tiles[st]):
            schedule.append((st, k == 0, k == st_tiles[st] - 1))

    per_core = []
    for c in range(NCORES):
        s_c, d_c, w_c, bounds = cores[c]
        src_pad = np.zeros(EC, np.int64)
        dstl_pad = np.full(EC, -1, np.int64)
        ewt_pad = np.zeros(EC, np.float32)
        pos = 0
        for st in range(NSEG):
            sl = slice(bounds[st], bounds[st + 1])
            n = bounds[st + 1] - bounds[st]
            # within a segtile the edge order is free (the one-hot encodes
            # each edge's dst slot): sort by src so the gather's table reads
            # are quasi-sequential (DRAM page locality)
            so = (np.argsort(s_c[sl], kind="stable") if SRCSORT
                  else np.arange(n))
            src_pad[pos:pos + n] = s_c[sl][so]
            dstl_pad[pos:pos + n] = (d_c[sl] - st * P)[so]
            ewt_pad[pos:pos + n] = w_c[sl][so]
            pos += int(st_tiles[st]) * P
        # remap src node id -> padded table row
        tbl_row = (src_pad // ND) * NDP + (src_pad % ND)
        gidx = np.tile(tbl_row.reshape(EC // 16, 16).T.astype(np.int16), (8, 1))
        ewt_t = ewt_pad.reshape(ntiles, P).T.astype(
            np.float16 if EDGE16 else np.float32)  # [128, ntiles]
        dstl2 = dstl_pad.reshape(ntiles, P).T                  # [128, ntiles]
        per_core.append({"gidx": np.ascontiguousarray(gidx),
                         "ewt": np.ascontiguousarray(ewt_t),
                         "dstl": np.ascontiguousarray(dstl2.astype(np.float32))})
    return per_core, schedule, ntiles


def _shardT(x, c):
    dt = np.float16 if NODE16 else np.float32
    sh = np.zeros((x.shape[1], NDP), dt)
    sh[:, :ND] = x[c * ND:(c + 1) * ND].T.astype(dt)
    return sh


# ---------------------------------------------------------------- device IR

def _build_program(schedules, ntiles_v2c, ntiles_c2v, no_collective=False,
                   no_gather=False, act_dve=False, nrep=1, abl=()):
    import concourse.bacc as bacc
    import concourse.mybir as mybir
    import concourse.tile as tile
    from concourse.masks import make_identity

    f32, f16, i16 = mybir.dt.float32, mybir.dt.float16, mybir.dt.int16
    f8 = mybir.dt.float8e4 if OHDT == "f8" else mybir.dt.float16
    fe = f16 if EDGE16 else f32   # edge pipeline dtype
    fn = f16 if NODE16 else f32   # node feature / MLP dtype
    AF = mybir.ActivationFunctionType
    OP = mybir.AluOpType

    nc = bacc.Bacc("TRN2", target_bir_lowering=False, debug=False,
                   num_devices=NCORES, num_swdge_queues=NQ,
                   dynamic_dma_scratch_size=32768)

    # ---------------- dram tensor declarations
    def din(name, shape, dt=f32):
        return nc.dram_tensor(name, shape, dt, kind="ExternalInput")

    ecv, ecc = ntiles_v2c * P, ntiles_c2v * P
    dirs = {}
    for d, ec in (("v2c", ecv), ("c2v", ecc)):
        dirs[d] = {
            "gidx": din(f"{d}_gidx", [P, ec // 16], i16),
            "ewt": din(f"{d}_ewt", [P, ec // P], fe),
            "dstl": din(f"{d}_dstl", [P, ec // P]),
            # one-hot tiles generated on device from dstl (one-time), then
            # streamed per conv exactly like the old host-built input
            "oh": nc.dram_tensor(f"{d}_ohg", [P, ec // P, P], f8,
                                 kind="Internal"),
            "lew": din(f"{d}_lew", [P, NL * HID], fe),
            "w1": din(f"{d}_w1", [NL, HID, 2 * HID], fn),
            "w2": din(f"{d}_w2", [NL, 2 * HID, HID], fn),
            "b1": din(f"{d}_b1", [2 * HID, NL]),
            "b2": din(f"{d}_b2", [HID, NL]),
            "b2leb": din(f"{d}_b2leb", [P, NL * HID], f16),
            "leb": din(f"{d}_leb", [HID, NL]),
        }
    enc = {}
    for t in ("vals", "cons"):
        enc[t] = {
            "xT": din(f"{t}_xT", [2, NDP], fn),
            "peT": din(f"{t}_peT", [8, NDP], fn),
            "peTn": din(f"{t}_peTn", [8, NDP], fn),
            "ew": din(f"{t}_enc_w", [2, HID // 2], fn),
            "eb": din(f"{t}_enc_b", [HID // 2, 1]),
            "pw1": din(f"{t}_pe_w1", [8, HID], fn),
            "pb1": din(f"{t}_pe_b1", [HID, 1]),
            "pw2": din(f"{t}_pe_w2", [HID, HID // 2], fn),
            "pb2": din(f"{t}_pe_b2", [HID // 2, 1]),
            "prw1": din(f"{t}_pred_w1", [HID, HID], fn),
            "prb1": din(f"{t}_pred_b1", [HID, 1]),
            "prw2": din(f"{t}_pred_w2", [HID, 1], fn),
            "prb2": din(f"{t}_pred_b2", [1, 1]),
        }
    pv_out = nc.dram_tensor("pv_out", [NL, NDP], f32, kind="ExternalOutput")
    pc_out = nc.dram_tensor("pc_out", [NL, NDP], f32, kind="ExternalOutput")

    # per-exchange internal tensors: 6 tables (XV0, XC1, XV1, XC2, XV2, XC3)
    tables = []
    cc_ins = []
    tablecs = []
    ftbl = f16 if TBLDT == "f16" else f32
    TW = P if TBLDT == "f16" else HID  # table row width (elements)
    for k in range(6):
        cc_ins.append(nc.dram_tensor(f"cc_in_{k}", [NDP, HID], ftbl, kind="Internal"))
        if TBLDT == "f16":
            tablecs.append(nc.dram_tensor(f"tablec_{k}", [TBL, HID], f16,
                                          kind="Internal", addr_space="Shared"))
            tables.append(nc.dram_tensor(f"table_{k}", [TBL, P], f16,
                                         kind="Internal"))
        else:
            tables.append(nc.dram_tensor(f"table_{k}", [TBL, HID], f32,
                                         kind="Internal", addr_space="Shared"))
            tablecs.append(tables[-1])

    RG = [list(range(NCORES))]

    with tile.TileContext(nc) as tc:
        from contextlib import ExitStack
        with ExitStack() as ctx:
            const = ctx.enter_context(tc.tile_pool(name="const", bufs=1))
            nodes = ctx.enter_context(tc.tile_pool(name="nodes", bufs=1))
            pseg = ctx.enter_context(tc.tile_pool(name="pseg", bufs=PSEGB, space="PSUM"))
            pmlp = ctx.enter_context(tc.tile_pool(name="pmlp", bufs=PMLPB, space="PSUM"))
            pmlp2 = ctx.enter_context(tc.tile_pool(name="pmlp2", bufs=PMLP2B, space="PSUM"))
            ptr = ctx.enter_context(tc.tile_pool(name="ptr", bufs=PTRB, space="PSUM"))

            def load_const(pool, dram, shape, dt=f32, tag=None, in_ap=None,
                           out_3d=None):
                t = pool.tile(shape, dt, tag=tag or dram.name, name="lc")
                out_ap = t[:] if out_3d is None else t[:].rearrange(
                    "k (l m) -> k l m", l=out_3d)
                nc.sync.dma_start(out=out_ap,
                                  in_=in_ap if in_ap is not None else dram[:])
                return t

            ident = const.tile([P, P], f16 if TBLDT == "f16" else f32,
                               tag="ident")
            make_identity(nc, ident[:])
            epsb = const.tile([HID, 1], f32, tag="epsb")
            nc.vector.memset(epsb[:], 1e-16)

            dsb = {}
            for d in ("v2c", "c2v"):
                dd = dirs[d]
                ec = ecv if d == "v2c" else ecc
                dsb[d] = {
                    "gidx": load_const(const, dd["gidx"], [P, ec // 16], i16),
                    "ewt": load_const(const, dd["ewt"], [P, ec // P], fe),
                    "lew": load_const(const, dd["lew"], [P, NL * HID], fe),
                    "w1": load_const(const, dd["w1"], [HID, NL * 2 * HID], fn,
                                     out_3d=NL,
                                     in_ap=dd["w1"][:].rearrange("l k m -> k l m")),
                    "w2": load_const(const, dd["w2"], [2 * HID, NL * HID], fn,
                                     out_3d=NL,
                                     in_ap=dd["w2"][:].rearrange("l k m -> k l m")),
                    "b1": load_const(const, dd["b1"], [2 * HID, NL]),
                    "b2": load_const(const, dd["b2"], [HID, NL]),
                    "b2leb": load_const(const, dd["b2leb"], [P, NL * HID], f16),
                    "leb": load_const(const, dd["leb"], [HID, NL]),
                    "oh_dram": dd["oh"],
                    "ntiles": ec // P,
                }
            esb = {}
            _f16keys = ("ew", "pw1", "pw2", "prw1", "prw2")
            for t in ("vals", "cons"):
                ee = enc[t]
                esb[t] = {k: load_const(const, ee[k], list(ee[k].shape),
                                        fn if k in _f16keys else f32,
                                        tag=f"{t}_{k}")
                          for k in ("ew", "eb", "pw1", "pb1", "pw2", "pb2",
                                    "prw1", "prb1", "prw2", "prb2")}

            NCHUNK = NDP // 512  # 5

            xv_ab = [nodes.tile([HID, NDP], fn, tag="xv_a", name="xv_a"),
                     nodes.tile([HID, NDP], fn, tag="xv_b", name="xv_b")]
            xc_ab = [nodes.tile([HID, NDP], fn, tag="xc_a", name="xc_a"),
                     nodes.tile([HID, NDP], fn, tag="xc_b", name="xc_b")]

            # ---------------- encoder (scoped input pool, freed afterwards
            # when nrep == 1; held open across reps otherwise)
            def load_eio(encio):
                # xT | peT | peTn per node type.  Matmul needs operands at a
                # common base partition, so each lives in its own tile; xT(2)
                # and peT/peTn(8) are concatenated along the free dim to share
                # one column where possible.
                eio = {}
                for t in ("vals", "cons"):
                    ee = enc[t]
                    et = encio.tile([8, 2 * NDP], fn, tag=f"{t}_pe2", name="pe2")
                    nc.sync.dma_start(out=et[:, 0:NDP], in_=ee["peT"][:])
                    nc.sync.dma_start(out=et[:, NDP:2 * NDP], in_=ee["peTn"][:])
                    xt = encio.tile([2, NDP], fn, tag=f"{t}_xt", name="xt")
                    nc.sync.dma_start(out=xt[:], in_=ee["xT"][:])
                    eio[t] = (xt, et)
                return eio

            def make_encoder(encio, eio):
                def encoder(t, out_tile):
                    e = esb[t]
                    xt, et = eio[t]
                    for ch in range(NCHUNK):
                        sl = slice(ch * 512, (ch + 1) * 512)
                        sln = slice(NDP + ch * 512, NDP + (ch + 1) * 512)
                        pm = pmlp.tile([HID // 2, 512], f32, tag="pm1", name="pm")
                        nc.tensor.matmul(pm[:], lhsT=e["ew"][:], rhs=xt[:, sl],
                                         start=True, stop=True)
                        nc.scalar.activation(out_tile[0:HID // 2, sl], pm[:],
                                             AF.Relu, bias=e["eb"][:])
                        hpe = encio.tile([HID, 512], fn, tag="hpe", name="hpe")
                        pp = pmlp2.tile([HID, 512], f32, tag="pm2", name="pp")
                        nc.tensor.matmul(pp[:], lhsT=e["pw1"][:], rhs=et[:, sl],
                                         start=True, stop=True)
                        nc.scalar.activation(hpe[:], pp[:], AF.Relu, bias=e["pb1"][:])
                        hpen = encio.tile([HID, 512], fn, tag="hpen", name="hpen")
                        ppn = pmlp2.tile([HID, 512], f32, tag="pm2", name="ppn")
                        nc.tensor.matmul(ppn[:], lhsT=e["pw1"][:],
                                         rhs=et[:, sln],
                                         start=True, stop=True)
                        nc.scalar.activation(hpen[:], ppn[:], AF.Relu,
                                             bias=e["pb1"][:])
                        p2 = pmlp.tile([HID // 2, 512], f32, tag="pm1", name="p2e")
                        nc.tensor.matmul(p2[:], lhsT=e["pw2"][:], rhs=hpe[:],
                                         start=True, stop=False)
                        nc.tensor.matmul(p2[:], lhsT=e["pw2"][:], rhs=hpen[:],
                                         start=False, stop=True)
                        nc.scalar.activation(out_tile[HID // 2:HID, sl], p2[:],
                                             AF.Relu, bias=e["pb2"][:], scale=0.5)
                return encoder

            if nrep == 1:
                with tc.tile_pool(name="encio", bufs=1) as encio:
                    encoder = make_encoder(encio, load_eio(encio))
                    encoder("vals", xv_ab[0])
                    encoder("cons", xc_ab[0])
            else:
                encio = ctx.enter_context(tc.tile_pool(name="encio", bufs=1))
                encoder = make_encoder(encio, load_eio(encio))

            # ---------------- one-time on-device one-hot generation
            # oh[e, t, d] = (dstl[e, t] == d).  v2c one-hots live resident in
            # SBUF (f8, ~61KB/partition) and never touch DRAM; c2v one-hots
            # are written to DRAM once per dispatch and streamed per conv.
            iota = const.tile([P, P], f16, tag="iota")
            nc.gpsimd.iota(iota[:], pattern=[[1, P]], base=0,
                           channel_multiplier=0,
                           allow_small_or_imprecise_dtypes=True)
            ohres = None
            gres = min(OHRESG, ecv // P // TPB) if OHRES else 0
            if OHRES:
                ohres = const.tile([P, gres * TPB * P], f8, tag="ohres")
            with tc.tile_pool(name="ohgen", bufs=4) as ohgen:
                dstl_sb = {
                    d: load_const(ohgen, dirs[d]["dstl"],
                                  [P, (ecv if d == "v2c" else ecc) // P],
                                  tag=f"{d}_dstl")
                    for d in ("v2c", "c2v")}
                if OHRES:
                    for ti in range(gres * TPB):
                        nc.gpsimd.tensor_scalar(
                            out=ohres[:, ti * P:(ti + 1) * P], in0=iota[:],
                            scalar1=dstl_sb["v2c"][:, ti:ti + 1],
                            scalar2=None, op0=OP.is_equal)
                for d in ("v2c", "c2v"):
                    sb = dsb[d]
                    ntl = sb["ntiles"]
                    g0 = gres if d == "v2c" else 0
                    for g in range(g0, ntl // TPB):
                        og = ohgen.tile([P, TPB * P], f8, tag="og", name="og")
                        og3 = og[:].rearrange("p (t f) -> p t f", f=P)
                        for t in range(TPB):
                            nc.gpsimd.tensor_scalar(
                                out=og3[:, t, :], in0=iota[:],
                                scalar1=dstl_sb[d][:, g * TPB + t:g * TPB + t + 1],
                                scalar2=None, op0=OP.is_equal)
                        nc.sync.dma_start(
                            out=dirs[d]["oh"][:, g * TPB:(g + 1) * TPB, :],
                            in_=og3)

            # ---------------- main pools
            stream = ctx.enter_context(tc.tile_pool(name="stream", bufs=SBUFS))
            work = ctx.enter_context(tc.tile_pool(name="work", bufs=WBUFS))
            wt = ctx.enter_context(tc.tile_pool(name="wt", bufs=1))
            outpre = nodes.tile([HID, NDP], fn, tag="outpre", name="outpre")
            nmt = nodes.tile([P, NSEG * HID], f16, tag="nmt", name="nmt")

            # ---------------- table write + exchange
            def write_table(src_tile, leb_col, k, nm_pre=False):
                """Node-major payload (prebuilt in nmt by conv_layer, or via
                transposes for the encoder) -> cc_in_k -> AllGather."""
                if "nowt" in abl:
                    return
                ftw = f16 if TBLDT == "f16" else f32
                if nm_pre:
                    nm = nmt
                else:
                    # encoder-output exchange: reuse outpre/nmt as scratch
                    # (both idle until the first conv runs)
                    tleb = outpre
                    nc.scalar.activation(tleb[:], src_tile[:], AF.Identity,
                                         bias=leb_col)
                    nm = nmt
                    for s in range(NSEG):
                        pt = ptr.tile([P, HID], ftw, tag="pt", name="pt")
                        nc.tensor.transpose(pt[:], tleb[:, s * P:(s + 1) * P],
                                            ident[:HID, :HID])
                        nc.vector.tensor_copy(nm[:, s * HID:(s + 1) * HID],
                                              pt[:])
                nc.sync.dma_start(
                    out=cc_ins[k][:].rearrange("(s p) f -> p s f", p=P),
                    in_=nm[:].rearrange("p (s f) -> p s f", f=HID))
                if no_collective:
                    nc.sync.dma_start(out=tables[k][0:NDP, 0:HID], in_=cc_ins[k][:])
                else:
                    nc.gpsimd.collective_compute(
                        "AllGather", OP.bypass,
                        ins=[cc_ins[k][:]], outs=[tablecs[k][:]],
                        replica_groups=RG)
                    if TBLDT == "f16":
                        nc.sync.dma_start(out=tables[k][:, 0:HID],
                                          in_=tablecs[k][:])

            # ---------------- one message-passing layer
            def conv_layer(d, i, x_dst, out_tile, table_in, make_nm=True):
                sb = dsb[d]
                sched = schedules[d]
                ntl = sb["ntiles"]
                ngroups = ntl // TPB
                lew_b = sb["lew"][:, i * HID:(i + 1) * HID].unsqueeze(1) \
                    .to_broadcast([P, TPB, HID])
                w1 = sb["w1"][:, i * 2 * HID:(i + 1) * 2 * HID]
                w2 = sb["w2"][:, i * HID:(i + 1) * HID]
                b2leb = sb["b2leb"][:, i * HID:(i + 1) * HID]

                def mlp_chunk(ch):
                    # MLP: out = W2^T relu(W1^T outpre + b1) + b2, interleaved
                    # into the group loop as soon as the 4 segtiles feeding
                    # chunk ch have closed (shortens the conv -> AllGather
                    # critical path).  Node-major table rows come from extra
                    # [h_chunk^T x W2] matmuls (no PE transposes -> the
                    # AllGather is not serialized against them).
                    sl = slice(ch * 512, (ch + 1) * 512)
                    p1 = pmlp.tile([2 * HID, 512], f32, tag="pm1", name="p1")
                    nc.tensor.matmul(p1[:], lhsT=w1, rhs=outpre[:, sl],
                                     start=True, stop=False)
                    nc.tensor.matmul(p1[:], lhsT=w1, rhs=x_dst[:, sl],
                                     start=False, stop=True)
                    h = work.tile([2 * HID, 512], fn, tag="h", name="h")
                    if act_dve:
                        nc.vector.tensor_scalar(out=h[:], in0=p1[:],
                                                scalar1=sb["b1"][:, i:i + 1],
                                                scalar2=0.0, op0=OP.add, op1=OP.max)
                    else:
                        nc.scalar.activation(h[:], p1[:], AF.Relu,
                                             bias=sb["b1"][:, i:i + 1])
                    p2 = pmlp2.tile([HID, 512], f32, tag="pm2", name="p2")
                    nc.tensor.matmul(p2[:], lhsT=w2, rhs=h[:],
                                     start=True, stop=True)
                    nc.scalar.activation(out_tile[:, sl], p2[:], AF.Identity,
                                         bias=sb["b2"][:, i:i + 1])
                    if make_nm:
                        for q in range(4):
                            sseg = ch * 4 + q
                            pn = ptr.tile([P, HID], f32, tag="pt", name="pn")
                            nc.tensor.matmul(
                                pn[:], lhsT=h[:, q * P:(q + 1) * P],
                                rhs=w2, start=True, stop=True)
                            nc.vector.tensor_tensor(
                                out=nmt[:, sseg * HID:(sseg + 1) * HID],
                                in0=pn[:], in1=b2leb, op=OP.add)

                segpsum = {}
                closed = 0
                next_chunk = 0
                ohc = None
                if "nooh" in abl:
                    ohc = wt.tile([P, TPB * P], f8, tag="ohc", name="ohc")
                    nc.sync.dma_start(out=ohc[:], in_=sb["oh_dram"][:, 0:TPB, :])
                gtc = None
                if "nogt" in abl:
                    gtw0 = P if TBLDT == "f16" else HID
                    gtc = wt.tile([P, TPB * gtw0], f16 if TBLDT == "f16" else f32,
                                  tag="gtc", name="gtc")
                    nc.sync.dma_start(
                        out=gtc[:].rearrange("p (t f) -> p t f", f=gtw0),
                        in_=table_in[0:GROUP, :].rearrange("(t p) f -> p t f", p=P))
                if "noseg" in abl:
                    nc.vector.memset(outpre[:], 0.0)
                for g in range(ngroups):
                    gtw = P if TBLDT == "f16" else HID
                    if "nogt" in abl:
                        gt = gtc
                        gt3 = gt[:].rearrange("p (t f) -> p t f", f=gtw)
                    else:
                        gt = stream.tile([P, TPB * gtw], f16 if TBLDT == "f16" else f32,
                                         tag="gather", name="gt")
                        gt3 = gt[:].rearrange("p (t f) -> p t f", f=gtw)
                    if "nogt" in abl:
                        pass
                    elif no_gather:
                        nc.sync.dma_start(
                            out=gt3,
                            in_=table_in[0:GROUP, :].rearrange(
                                "(t p) f -> p t f", p=P))
                    else:
                        nc.gpsimd.dma_gather(
                            gt3, table_in[:],
                            sb["gidx"][:, g * (GROUP // 16):(g + 1) * (GROUP // 16)],
                            num_idxs=GROUP, num_idxs_reg=GROUP, elem_size=gtw,
                            single_packet=SPK, queue_num=g % NQ)
                    if "nooh" in abl:
                        oh = ohc
                    elif OHRES and d == "v2c" and g < gres:
                        oh = None
                    else:
                        oh = stream.tile([P, TPB * P], f8, tag="oh", name="oh")
                        nc.sync.dma_start(
                            out=oh[:],
                            in_=sb["oh_dram"][:, g * TPB:(g + 1) * TPB, :])
                    ewt_b = sb["ewt"][:, g * TPB:(g + 1) * TPB].to_broadcast(
                        [P, TPB, HID])
                    if "noedge" in abl:
                        v16 = gt
                        v3 = v16[:].rearrange("p (t f) -> p t f", f=P)
                    else:
                        cm = work.tile([P, TPB * HID], fe, tag="cm", name="cm")
                        cm3 = cm[:].rearrange("p (t f) -> p t f", f=HID)
                        nc.vector.tensor_tensor(out=cm3, in0=ewt_b, in1=lew_b,
                                                op=OP.mult)
                        m0 = work.tile([P, TPB * HID], fe, tag="m0", name="m0")
                        m3 = m0[:].rearrange("p (t f) -> p t f", f=HID)
                        nc.vector.tensor_tensor(out=m3, in0=cm3,
                                                in1=gt3[:, :, 0:HID], op=OP.add)

                        r16 = work.tile([P, TPB * HID], f16, tag="r16", name="r16")
                        nc.scalar.activation(r16[:], m0[:], AF.Relu)
                        v16 = stream.tile([P, TPB * P], f16, tag="v16", name="v16")
                        v3 = v16[:].rearrange("p (t f) -> p t f", f=P)
                        r3 = r16[:].rearrange("p (t f) -> p t f", f=HID)
                        nc.scalar.activation(v3[:, :, 0:HID], r3, AF.Exp)
                        nc.vector.tensor_tensor(out=v3[:, :, HID:P],
                                                in0=v3[:, :, 0:HID], in1=r3,
                                                op=OP.mult)
                    if oh is None:
                        oh3 = ohres[:, g * TPB * P:(g + 1) * TPB * P].rearrange(
                            "p (t f) -> p t f", f=P)
                    else:
                        oh3 = oh[:].rearrange("p (t f) -> p t f", f=P)
                    for t in range(TPB):
                        gt_i = g * TPB + t
                        st, is_start, is_stop = sched[gt_i]
                        if "noscatter2" in abl:
                            continue
                        if is_start:
                            segpsum[st] = pseg.tile([P, P], f32, tag="seg",
                                                    name="segps")
                        if "noscatter" in abl and not (is_start or is_stop):
                            continue
                        nc.tensor.matmul(segpsum[st][:],
                                         lhsT=v3[:, t, :], rhs=oh3[:, t, :],
                                         start=is_start, stop=is_stop)
                        if is_stop:
                            ps = segpsum.pop(st)
                            if "noseg" in abl:
                                continue
                            sl = slice(st * P, (st + 1) * P)
                            sg = work.tile([HID, P], f32, tag="sg", name="sg")
                            nc.scalar.activation(sg[:], ps[0:HID, :],
                                                 AF.Identity, bias=epsb[:])
                            rec = work.tile([HID, P], f32, tag="rec", name="rec")
                            nc.vector.reciprocal(rec[:], sg[:])
                            nc.vector.tensor_tensor(out=outpre[:, sl],
                                                    in0=ps[HID:P, :],
                                                    in1=rec[:], op=OP.mult)
                            closed += 1
                            if "nomlp" not in abl:
                                while (next_chunk < NCHUNK
                                       and closed >= 4 * (next_chunk + 1)):
                                    mlp_chunk(next_chunk)
                                    next_chunk += 1
                if "nomlp" in abl:
                    nc.vector.memset(out_tile[:], 0.0)
                    if make_nm:
                        nc.vector.memset(nmt[:], 0.0)
                else:
                    while next_chunk < NCHUNK:
                        mlp_chunk(next_chunk)
                        next_chunk += 1

            # ---------------- prediction head (inline per layer)
            def pred_head(t, i, h_tile, out_dram):
                e = esb[t]
                if "nopred" in abl:
                    po = work.tile([1, 512], f32, tag="po", name="po")
                    nc.vector.memset(po[:], 0.0)
                    nc.sync.dma_start(out=out_dram[i:i + 1, 0:512], in_=po[:])
                    return
                for ch in range(NCHUNK):
                    sl = slice(ch * 512, (ch + 1) * 512)
                    p1 = pmlp.tile([HID, 512], f32, tag="pm1", name="pp1")
                    nc.tensor.matmul(p1[:], lhsT=e["prw1"][:], rhs=h_tile[:, sl],
                                     start=True, stop=True)
                    ph = work.tile([2 * HID, 512], fn, tag="h", name="ph")
                    if act_dve:
                        nc.vector.tensor_scalar(out=ph[:HID, :], in0=p1[:],
                                                scalar1=e["prb1"][:],
                                                scalar2=0.0, op0=OP.add, op1=OP.max)
                    else:
                        nc.scalar.activation(ph[:HID, :], p1[:], AF.Relu,
                                             bias=e["prb1"][:])
                    p2 = pmlp2.tile([1, 512], f32, tag="pm2", name="pp2")
                    nc.tensor.matmul(p2[:], lhsT=e["prw2"][:], rhs=ph[:HID, :],
                                     start=True, stop=True)
                    po = work.tile([1, 512], f32, tag="po", name="po")
                    nc.vector.tensor_scalar(out=po[:], in0=p2[:],
                                            scalar1=e["prb2"][:],
                                            scalar2=None, op0=OP.add)
                    nc.sync.dma_start(out=out_dram[i:i + 1, sl], in_=po[:])

            # ---------------- main sequence
            # exchange k: 0=XV0, 1=XC1, 2=XV1, 3=XC2, 4=XV2, 5=XC3
            # nrep > 1 (bench only): run the whole computation nrep times
            # back-to-back; every rep re-encodes from the resident inputs, so
            # each rep is a complete, identical execution and the outputs of
            # the last rep are the (correct) program outputs.
            for rep in range(nrep):
                if nrep > 1 and ("noenc" not in abl or rep == 0):
                    encoder("vals", xv_ab[0])
                    encoder("cons", xc_ab[0])
                write_table(xv_ab[0], dsb["v2c"]["leb"][:, 0:1], 0)
                for i in range(NL):
                    new_xc = xc_ab[(i + 1) % 2]
                    conv_layer("v2c", i, xc_ab[i % 2], new_xc, tables[2 * i])
                    k = 2 * i + 1
                    write_table(None, None, k, nm_pre=True)
                    pred_head("cons", i, new_xc, pc_out)
                    new_xv = xv_ab[(i + 1) % 2]
                    conv_layer("c2v", i, xv_ab[i % 2], new_xv, tables[k],
                               make_nm=(i < NL - 1))
                    if i < NL - 1:
                        write_table(None, None, 2 * (i + 1), nm_pre=True)
                    pred_head("vals", i, new_xv, pv_out)

    nc.compile()
    return nc


# ---------------------------------------------------------------- entry

def kernel(**inputs):
    from concourse.bass_utils import run_bass_kernel_spmd

    inp = {k: np.asarray(v) for k, v in inputs.items()}

    v2c_cores, v2c_sched, ntv = _prep_direction(
        inp["edge_index_v2c"][0], inp["edge_index_v2c"][1], inp["edge_weight_v2c"])
    c2v_cores, c2v_sched, ntc = _prep_direction(
        inp["edge_index_c2v"][0], inp["edge_index_c2v"][1], inp["edge_weight_c2v"])

    import os as _os
    _nocc = bool(_os.environ.get("GNN_NOCC"))
    _nogather = bool(_os.environ.get("GNN_NOGATHER"))
    _actdve = _os.environ.get("GNN_ACTDVE", "0") != "0"
    _nrep = int(_os.environ.get("GNN_NREP", "1"))
    _abl = tuple(x for x in _os.environ.get("GNN_ABL", "").split(",") if x)
    key = (ntv, ntc, _nocc, _nogather, _actdve, _nrep, _abl, TPB, NQ, SBUFS, WBUFS, PSEGB, PMLPB, PMLP2B, PTRB, SPK, OHDT, OHRES, OHRESG, TBLDT, EDGE16, NODE16,
           tuple(s[0] for s in v2c_sched), tuple(s[0] for s in c2v_sched))
    if key not in _PROG_CACHE:
        _PROG_CACHE[key] = _build_program(
            {"v2c": v2c_sched, "c2v": c2v_sched}, ntv, ntc,
            no_collective=_nocc, no_gather=_nogather, act_dve=_actdve,
            nrep=_nrep, abl=_abl)
    nc = _PROG_CACHE[key]

    # ---- shared (replicated) weight tensors
    shared = {}
    for d in ("v2c", "c2v"):
        lew = inp[f"{d}_edge_w"][:, 0, :]            # [NL, HID]
        shared[f"{d}_lew"] = np.tile(lew.reshape(1, NL * HID), (P, 1)).astype(
            np.float16 if EDGE16 else np.float32)
        w1 = inp[f"{d}_w1"].astype(np.float32)       # [NL, HID, 2H]
        b1 = inp[f"{d}_b1"].astype(np.float32)       # [NL, 2H]
        # fold msg eps: out_pre_true = out_pre + EPS (per feature, all features)
        # -> b1' = b1 + EPS * sum_f w1[f, :]
        b1p = b1 + EPS * w1.sum(axis=1)
        _ndt = np.float16 if NODE16 else np.float32
        shared[f"{d}_w1"] = w1.astype(_ndt)
        shared[f"{d}_w2"] = inp[f"{d}_w2"].astype(_ndt)
        shared[f"{d}_b1"] = np.ascontiguousarray(b1p.T)          # [2H, NL]
        shared[f"{d}_b2"] = np.ascontiguousarray(inp[f"{d}_b2"].T)  # [H, NL]
        shared[f"{d}_leb"] = np.ascontiguousarray(inp[f"{d}_edge_b"].T)  # [H, NL]
    # folded (own b2 + consumer-layer edge bias) broadcast rows, node-major
    v2c_b2 = inp["v2c_b2"].astype(np.float32)   # [NL, H]
    c2v_b2 = inp["c2v_b2"].astype(np.float32)
    v2c_leb = inp["v2c_edge_b"].astype(np.float32)  # [NL, H]
    c2v_leb = inp["c2v_edge_b"].astype(np.float32)
    bl_v = np.zeros((NL, HID), np.float32)  # v2c conv i -> XC table k=2i+1
    bl_c = np.zeros((NL, HID), np.float32)  # c2v conv i -> XV table k=2i+2
    for i in range(NL):
        bl_v[i] = v2c_b2[i] + c2v_leb[i]
        if i < NL - 1:
            bl_c[i] = c2v_b2[i] + v2c_leb[i + 1]
    shared["v2c_b2leb"] = np.tile(bl_v.reshape(1, NL * HID),
                                  (P, 1)).astype(np.float16)
    shared["c2v_b2leb"] = np.tile(bl_c.reshape(1, NL * HID),
                                  (P, 1)).astype(np.float16)
    for t in ("vals", "cons"):
        _ndt = np.float16 if NODE16 else np.float32
        shared[f"{t}_enc_w"] = inp[f"enc_{t}_w"].astype(_ndt)
        shared[f"{t}_enc_b"] = inp[f"enc_{t}_b"].reshape(-1, 1).astype(np.float32)
        shared[f"{t}_pe_w1"] = inp[f"pe_{t}_w1"].astype(_ndt)
        shared[f"{t}_pe_b1"] = inp[f"pe_{t}_b1"].reshape(-1, 1).astype(np.float32)
        shared[f"{t}_pe_w2"] = inp[f"pe_{t}_w2"].astype(_ndt)
        shared[f"{t}_pe_b2"] = inp[f"pe_{t}_b2"].reshape(-1, 1).astype(np.float32)
        shared[f"{t}_pred_w1"] = inp[f"pred_{t}_w1"].astype(_ndt)
        shared[f"{t}_pred_b1"] = inp[f"pred_{t}_b1"].reshape(-1, 1).astype(np.float32)
        shared[f"{t}_pred_w2"] = inp[f"pred_{t}_w2"].astype(_ndt)
        shared[f"{t}_pred_b2"] = inp[f"pred_{t}_b2"].reshape(-1, 1).astype(np.float32)

    in_maps = []
    for c in range(NCORES):
        m = dict(shared)
        for d, cores in (("v2c", v2c_cores), ("c2v", c2v_cores)):
            m[f"{d}_gidx"] = cores[c]["gidx"]
            m[f"{d}_ewt"] = cores[c]["ewt"]
            m[f"{d}_dstl"] = cores[c]["dstl"]
        for t, x, pe in (("vals", inp["x_vals"], inp["pe_vals"]),
                         ("cons", inp["x_cons"], inp["pe_cons"])):
            m[f"{t}_xT"] = _shardT(x, c)
            peT = _shardT(pe, c)
            m[f"{t}_peT"] = peT
            m[f"{t}_peTn"] = -peT
        in_maps.append(m)

    import os
    global LAST_EXEC_NS
    nbench = int(os.environ.get("GNN_BENCH", "0"))
    if nbench:
        results, LAST_EXEC_NS = _run_benched(nc, in_maps, nbench)
    elif os.environ.get("GNN_SIM"):
        from concourse.bass_interp import MultiCoreSim
        sim = MultiCoreSim(nc, num_cores=NCORES, num_workers=8)
        for c, cs in sim.cores.items():
            for k, v in in_maps[c].items():
                cs.tensor(k)[:] = v
        sim.simulate(check_with_hw=False)
        results = [{k: np.asarray(sim.cores[c].tensor(k))
                    for k in ("pv_out", "pc_out")} for c in range(NCORES)]
    else:
        res = run_bass_kernel_spmd(nc, in_maps, core_ids=list(range(NCORES)))
        LAST_EXEC_NS = res.exec_time_ns
        results = res.results

    pv = np.zeros((NV, NL), np.float32)
    pc = np.zeros((NC, NL), np.float32)
    for c in range(NCORES):
        pv[c * ND:(c + 1) * ND] = results[c]["pv_out"][:, :ND].T
        pc[c * ND:(c + 1) * ND] = results[c]["pc_out"][:, :ND].T
    return pv, pc


LAST_EXEC_NS = None
LAST_FLOOR_NS = None
_FLOOR_PROG = None


def _floor_prog():
    """Trivial 8-core program used to measure the per-dispatch overhead."""
    global _FLOOR_PROG
    if _FLOOR_PROG is None:
        import concourse.bacc as bacc
        import concourse.mybir as mybir
        import concourse.tile as tile
        f32 = mybir.dt.float32
        fnc = bacc.Bacc("TRN2", target_bir_lowering=False, debug=False,
                        num_devices=NCORES)
        xin = fnc.dram_tensor("xin", [P, P], f32, kind="ExternalInput")
        xout = fnc.dram_tensor("xout", [P, P], f32, kind="ExternalOutput")
        with tile.TileContext(fnc) as tc:
            with tc.tile_pool(name="p", bufs=1) as pool:
                t = pool.tile([P, P], f32)
                fnc.sync.dma_start(out=t[:], in_=xin[:])
                fnc.sync.dma_start(out=xout[:], in_=t[:])
        fnc.compile()
        _FLOOR_PROG = fnc
    return _FLOOR_PROG


def _run_benched(nc, in_maps, niter):
    """Bench the main program (GNN_NREP executions per dispatch, amortized);
    optionally also the trivial floor program (dispatch-overhead calibration).
    Returns per-execution time."""
    import os
    global LAST_FLOOR_NS
    nrep = int(os.environ.get("GNN_NREP", "1"))
    results, ns_dispatch = _bench_once(nc, in_maps, niter)
    ns = ns_dispatch // nrep
    print(f"[bench] per-exec (dispatch/{nrep}): {ns/1e6:.3f} ms")
    if os.environ.get("GNN_FLOOR", "1") != "0":
        fnc = _floor_prog()
        fmaps = [{"xin": np.zeros((P, P), np.float32)} for _ in range(NCORES)]
        _, fns = _bench_once(fnc, fmaps, niter)
        LAST_FLOOR_NS = fns
        print(f"[bench] floor: {fns/1e6:.3f} ms/dispatch; "
              f"device-only estimate: {(ns_dispatch - fns)/nrep/1e6:.3f} ms/exec")
    return results, ns


def _bench_once(nc, in_maps, niter):
    """Compile once via the bass2jax PJRT path, then time `niter` executions
    with device-resident inputs. Returns (results, per-iter exec ns)."""
    import os
    import time
    import jax
    import jax.numpy as jnp
    from jax.sharding import Mesh, PartitionSpec
    from jax.experimental.shard_map import shard_map
    import concourse.mybir as mybir
    from concourse import bass2jax

    bass2jax.install_neuronx_cc_hook()
    partition_name = nc.partition_id_tensor.name if nc.partition_id_tensor else None
    in_names, out_names, out_avals = [], [], []
    for alloc in nc.m.functions[0].allocations:
        if not isinstance(alloc, mybir.MemoryLocationSet):
            continue
        name = alloc.memorylocations[0].name
        if alloc.kind == "ExternalInput":
            if name != partition_name:
                in_names.append(name)
        elif alloc.kind == "ExternalOutput":
            out_names.append(name)
            out_avals.append(jax.core.ShapedArray(
                tuple(alloc.tensor_shape), mybir.dt.np(alloc.dtype)))
    n_params = len(in_names)
    all_in_names = in_names + out_names
    if partition_name is not None:
        all_in_names = all_in_names + [partition_name]

    import jax.numpy as _jnp

    def _call_once(ins, zeros_ops):
        operands = list(ins) + list(zeros_ops)
        if partition_name is not None:
            operands.append(bass2jax.partition_id_tensor())
        outs = bass2jax._bass_exec_p.bind(
            *operands,
            out_avals=tuple(out_avals),
            in_names=tuple(all_in_names),
            out_names=tuple(out_names),
            lowering_input_output_aliases=(),
            sim_require_finite=True,
            sim_require_nnan=True,
            nc=nc,
        )
        return tuple(outs)

    def _make_body(nloop):
        def _body(*args):
            ins = args[:n_params]
            zeros_ops = args[n_params:]
            outs = _call_once(ins, zeros_ops)
            for _ in range(nloop - 1):
                zeros_ops = tuple(o * 0 for o in outs)
                outs = _call_once(ins, zeros_ops)
            return outs
        return _body
    _body = _make_body(1)
    NLOOP = int(os.environ.get("GNN_NLOOP", "1"))

    devices = jax.devices()[:NCORES]
    mesh = Mesh(np.asarray(devices), ("core",))
    n_outs = len(out_names)
    in_specs = (PartitionSpec("core"),) * (n_params + n_outs)
    out_specs = (PartitionSpec("core"),) * n_outs
    def make_sharded(nloop):
        return jax.jit(
            shard_map(_make_body(nloop), mesh=mesh, in_specs=in_specs,
                      out_specs=out_specs, check_rep=False),
            donate_argnums=tuple(range(n_params, n_params + n_outs)),
            keep_unused=True)
    sharded = make_sharded(NLOOP)

    from jax.sharding import NamedSharding
    shard = NamedSharding(mesh, PartitionSpec("core"))
    dev_in = []
    for i, name in enumerate(in_names):
        cat = np.concatenate([np.asarray(in_maps[c][name]) for c in range(NCORES)],
                             axis=0)
        dev_in.append(jax.device_put(cat, shard))

    def zeros():
        return [jax.device_put(
            np.zeros((NCORES * a.shape[0], *a.shape[1:]), a.dtype), shard)
            for a in out_avals]

    # warmup (compiles)
    out = sharded(*dev_in, *zeros())
    jax.block_until_ready(out)

    def timed(fn, reps=3):
        best = float("inf")
        for _ in range(reps):
            z = zeros()
            jax.block_until_ready(z)
            t0 = time.perf_counter()
            o = fn(*dev_in, *z)
            jax.block_until_ready(o)
            best = min(best, time.perf_counter() - t0)
        return best

    # async sequential loop: per-iter amortized time (dispatch latency
    # pipelines).  With GNN_NLOOP>1 each dispatch executes the NEFF nloop
    # times back-to-back on device, amortizing the fixed axon dispatch
    # overhead out of the per-execution number.
    zs = [zeros() for _ in range(niter)]
    jax.block_until_ready(zs)
    t0 = time.perf_counter()
    outs = None
    for k in range(niter):
        outs = sharded(*dev_in, *zs[k])
    jax.block_until_ready(outs)
    dt = (time.perf_counter() - t0) / (niter * NLOOP)
    print(f"[bench] async loop x{niter} (nloop={NLOOP}): "
          f"{dt*1e3:.3f} ms/exec")
    exec_ns = int(dt * 1e9)
    out = outs
    results = []
    for c in range(NCORES):
        results.append({
            name: np.asarray(out[i]).reshape(NCORES, *out_avals[i].shape)[c]
            for i, name in enumerate(out_names)})
    return results, exec_ns

